# revision 1
# baseline (speedup 1.0000x reference)
"""Box-attention kernel for Trainium2 (Bass/Tile), SPMD over 8 NeuronCores.

Problem: per-(batch, h, w) pixel attention over 32 boxes:
  scores[i,j] = <q[i,:,p], k[j,:,p]> / 8 ;  w = softmax_j ;  delta[i,c,p] = sum_j w[i,j] v[j,c,p]

Sharding: core = 2*b_half... each core owns (b = core//2, h in [40*(core%2), +40)).
Zero communication (pixel-parallel).

Per-core layout strategy (all f32):
 - chunk = 4 consecutive h rows x 80 w = 320 pixels.
 - q/k SBUF tiles: [part = 64*(h%2) + c][hp = h//2 % 2, box, w]
 - v SBUF tile:    [part = 32*(h%4) + j][c, w]
 - m1 (scores^T) per pixel: lhsT=k[c,j], rhs=q[c,i] -> e_psum[32g+j, 128*wl+32g+i]
   (g = h%4, wl = w%4; 16 pixels fill one PSUM bank [128,512], block-diagonal)
 - exp on whole bank (ACT, scale=1/8); denominator via one block-diag-ones matmul
   (sums each 32-partition group); reciprocal + multiply on DVE -> softmax weights W.
 - m2 per pixel: lhsT=W[j,i] (32x32 diag block), rhs=v[j,c] -> delta[32g+i, 64q+c]
   32 pixels per out bank; DVE copy to SBUF; DMA out.
All matmuls use explicit tile_position so 16 32x32 PE sub-arrays run concurrently.
"""

import sys

import numpy as np

try:
    import concourse.bass as bass
except ImportError:  # fresh grading dir: point at the in-container repo
    for p in ("/opt/trn_rl_repo", "/root/.axon_site/_ro/trn_rl_repo"):
        if p not in sys.path:
            sys.path.insert(0, p)
    import concourse.bass as bass

from contextlib import ExitStack

import concourse.bacc as bacc
import concourse.tile as tile
from concourse import mybir
from concourse.bass_utils import run_bass_kernel_spmd

NB, B, C, H, W = 32, 4, 64, 80, 80
HPC = H // 2  # h rows per core (8 cores = 4 batches x 2 h-halves)
CHH = 4  # chunk height (h rows)
NCHUNK = HPC // CHH
DT = mybir.dt.float32

_CACHE = {}


def build_nc(reps=1):
    nc = bacc.Bacc()
    q = nc.declare_dram_parameter("q", [NB, C, HPC, W], DT, isOutput=False)
    k = nc.declare_dram_parameter("k", [NB, C, HPC, W], DT, isOutput=False)
    v = nc.declare_dram_parameter("v", [NB, C, HPC, W], DT, isOutput=False)
    o = nc.declare_dram_parameter("o", [NB, C, HPC, W], DT, isOutput=True)

    # DRAM views matching the SBUF tile iteration orders.
    qv = q[:].rearrange("i c (hb hp h2) w -> hb h2 c hp i w", hp=2, h2=2)
    kv = k[:].rearrange("j c (hb hp h2) w -> hb h2 c hp j w", hp=2, h2=2)
    vv = v[:].rearrange("j c (hb h4) w -> hb h4 j c w", h4=4)
    ov = o[:].rearrange("i c (hb h4) w -> hb h4 i c w", h4=4)

    with tile.TileContext(nc) as tc, ExitStack() as ctx:
        io = ctx.enter_context(tc.tile_pool(name="io", bufs=2))
        sm = ctx.enter_context(tc.tile_pool(name="sm", bufs=4))
        cst = ctx.enter_context(tc.tile_pool(name="cst", bufs=1))
        ep = ctx.enter_context(tc.tile_pool(name="ep", bufs=1, space="PSUM"))
        dp = ctx.enter_context(tc.tile_pool(name="dp", bufs=2, space="PSUM"))
        op = ctx.enter_context(tc.tile_pool(name="op", bufs=3, space="PSUM"))

        # Block-diagonal ones [128,128]: sums each 32-partition group.
        ones = cst.tile([128, 128], DT)
        nc.vector.memset(ones, 0.0)
        for g in range(4):
            nc.vector.memset(ones[32 * g : 32 * g + 32, 32 * g : 32 * g + 32], 1.0)

        # Two persistent PSUM score banks; off-diagonal blocks stay zero forever.
        ebanks = []
        for t in range(3):
            eb = ep.tile([128, 512], DT, tag=f"eb{t}")
            nc.vector.memset(eb, 0.0)
            ebanks.append(eb)
        # Dummy matmuls: absorb the memset->PE waits here so the first real
        # matmul of the pipeline carries at most 2 sem waits (S3_LW limit).
        for t in range(3):
            nc.tensor.matmul(
                out=ebanks[t][0:32, 0:32],
                lhsT=ones[0:32, 0:32],
                rhs=ones[0:32, 0:32],
                start=True,
                stop=True,
                tile_position=(0, 0),
            )

        gidx = 0
        for hb in [h for _ in range(reps) for h in range(NCHUNK)]:
            tq = io.tile([128, 2, 32, W], DT, tag="tq")
            tk = io.tile([128, 2, 32, W], DT, tag="tk")
            tv = io.tile([128, C, W], DT, tag="tv")
            to = io.tile([128, C, W], DT, tag="to")
            tqv = tq.rearrange("(h2 c) hp i w -> h2 c hp i w", h2=2)
            tkv = tk.rearrange("(h2 c) hp j w -> h2 c hp j w", h2=2)
            for hp in range(2):
                for h2 in range(2):
                    nc.sync.dma_start(out=tqv[h2, :, hp], in_=qv[hb, h2, :, hp])
                    nc.sync.dma_start(out=tkv[h2, :, hp], in_=kv[hb, h2, :, hp])
            tvv = tv.rearrange("(h4 j) c w -> h4 j c w", h4=4)
            for h4 in range(4):
                nc.sync.dma_start(out=tvv[h4], in_=vv[hb, h4])

            ob = None
            for t in range(W // 4):  # 16-pixel groups: w in [4t, 4t+4)
                eb = ebanks[gidx % 3]
                gidx += 1
                for wl in range(4):
                    w_ = 4 * t + wl
                    for g in range(4):
                        par, hp = g % 2, g // 2
                        nc.tensor.matmul(
                            out=eb[
                                32 * g : 32 * g + 32,
                                128 * wl + 32 * g : 128 * wl + 32 * g + 32,
                            ],
                            lhsT=tk[64 * par : 64 * par + 64, hp, :, w_],
                            rhs=tq[64 * par : 64 * par + 64, hp, :, w_],
                            start=True,
                            stop=True,
                            tile_position=(64 * par, 32 * g),
                        )
                E = sm.tile([128, 512], DT, tag="E")
                nc.scalar.activation(
                    E, eb, mybir.ActivationFunctionType.Exp, scale=0.125
                )
                dn = dp.tile([128, 512], DT, tag="dn")
                nc.tensor.matmul(
                    out=dn, lhsT=ones, rhs=E, start=True, stop=True,
                    tile_position=(0, 0),
                )
                R = sm.tile([128, 512], DT, tag="R")
                nc.vector.reciprocal(R, dn)
                Wt = sm.tile([128, 512], DT, tag="Wt")
                nc.vector.tensor_mul(Wt, E, R)

                if t % 2 == 0:
                    ob = op.tile([128, 512], DT, tag="ob")
                for wl in range(4):
                    w_ = 4 * t + wl
                    q_ = 4 * (t % 2) + wl
                    for g in range(4):
                        nc.tensor.matmul(
                            out=ob[32 * g : 32 * g + 32, 64 * q_ : 64 * q_ + 64],
                            lhsT=Wt[
                                32 * g : 32 * g + 32,
                                128 * wl + 32 * g : 128 * wl + 32 * g + 32,
                            ],
                            rhs=tv[32 * g : 32 * g + 32, :, w_],
                            start=True,
                            stop=True,
                            tile_position=(32 * g, 32 * g),
                        )
                if t % 2 == 1:
                    w0 = 4 * (t - 1)
                    nc.vector.tensor_copy(
                        out=to[:, :, w0 : w0 + 8].rearrange("p c w -> p w c"),
                        in_=ob.rearrange("p (q c) -> p q c", q=8),
                    )
            tov = to.rearrange("(h4 i) c w -> h4 i c w", h4=4)
            for h4 in range(4):
                nc.sync.dma_start(out=ov[hb, h4], in_=tov[h4])
    nc.compile()
    return nc


def _get_nc(reps=1):
    key = f"nc{reps}"
    if key not in _CACHE:
        _CACHE[key] = build_nc(reps)
    return _CACHE[key]


def kernel(q_big, k_big, v_big, **run_kwargs):
    q_big = np.asarray(q_big, dtype=np.float32)
    k_big = np.asarray(k_big, dtype=np.float32)
    v_big = np.asarray(v_big, dtype=np.float32)
    nc = _get_nc()
    in_maps = []
    for core in range(8):
        b, h0 = core // 2, HPC * (core % 2)
        in_maps.append(
            {
                "q": np.ascontiguousarray(q_big[:, b, :, h0 : h0 + HPC, :]),
                "k": np.ascontiguousarray(k_big[:, b, :, h0 : h0 + HPC, :]),
                "v": np.ascontiguousarray(v_big[:, b, :, h0 : h0 + HPC, :]),
            }
        )
    res = run_bass_kernel_spmd(nc, in_maps, list(range(8)), **run_kwargs)
    out = np.empty((NB, B, C, H, W), np.float32)
    for core in range(8):
        b, h0 = core // 2, HPC * (core % 2)
        out[:, b, :, h0 : h0 + HPC, :] = res.results[core]["o"]
    if run_kwargs:
        kernel.last_results = res
    return out



# revision 3
# speedup vs baseline: 2.9363x; 2.9363x over previous
"""Box-attention kernel for Trainium2 (Bass/Tile), SPMD over 8 NeuronCores.

Problem: per-(batch, h, w) pixel attention over 32 boxes:
  S[i,j] = <q[i,:,p], k[j,:,p]>/8 ; W = softmax_j S ; delta[i,c,p] = sum_j W[i,j] v[j,c,p]

Sharding: core = 2*b + h_half; each core owns (b = core//2, h in [40*(core%2), +40)).
Zero communication (pixel-parallel).

v2 design (vs v1 baseline at 1.245 ms):
 - fp16 I/O: host casts q,k,v to fp16 and the output back to fp32. Halves HBM
   traffic (52.4 MB/core); fp16 error (2^-11) is far inside the 2e-2 gate.
 - chunk = 8 h-rows; q/k SBUF [64*(hc//4)+c][i, hc%4, w] -> 640B DMA lines;
   v/out [32*(hc//2)+{j,i}][c, hc%2, w] -> 320B lines.
 - compact score banks: e-bank [128, 512] holds 64 pixels (4 part-groups x
   16 col-groups of 32) with zero padding waste. One exp per 64 px.
 - denominator fused into the delta matmul: v tile has a 65th channel == 1.0,
   so out[.., 64] = sum_j E[j,i] = denom. No ones-matmul, no [128,512]
   reciprocal. Reciprocal runs on [128,16] compact denominators instead
   (DVE iterative divide is ~6.5 cyc/elem -- was 53% of v1's span).
 - normalize+evacuate fused: one DVE tensor_mult per ob bank with rden
   broadcast (step-0) along c, writing fp16 straight into the staging tile.
 - PSUM: eb x2, (ob0,ob1,ob2) x2 = 8 banks exactly.
 - emission skew: scores(n+1) issued before delta(n) so the PE never waits
   on the ACT exp; engines pipeline across supergroups.
"""

import sys

import numpy as np

try:
    import concourse.bass as bass
except ImportError:  # fresh grading dir: point at the in-container repo
    for p in ("/opt/trn_rl_repo", "/root/.axon_site/_ro/trn_rl_repo"):
        if p not in sys.path:
            sys.path.insert(0, p)
    import concourse.bass as bass

from contextlib import ExitStack

import concourse.bacc as bacc
import concourse.tile as tile
from concourse import mybir
from concourse.bass_utils import run_bass_kernel_spmd

NB, B, C, H, W = 32, 4, 64, 80, 80
HPC = H // 2  # h rows per core (8 cores = 4 batches x 2 h-halves)
CHH = 8  # chunk height (h rows)
NCHUNK = HPC // CHH  # 5
WB = 16  # w values per supergroup (64 px = 4 groups x 16 cols)
NSGW = W // WB  # 5 supergroups per (chunk, r)
F16 = mybir.dt.float16
F32 = mybir.dt.float32

_CACHE = {}


def build_nc():
    nc = bacc.Bacc()
    q = nc.declare_dram_parameter("q", [NB, C, HPC, W], F16, isOutput=False)
    k = nc.declare_dram_parameter("k", [NB, C, HPC, W], F16, isOutput=False)
    v = nc.declare_dram_parameter("v", [NB, C, HPC, W], F16, isOutput=False)
    o = nc.declare_dram_parameter("o", [NB, C, HPC, W], F16, isOutput=True)

    # DRAM views: h = CHH*hb + hc; q/k use s = hc//4 (640B lines), v/o use
    # g = hc//2 (320B lines).
    qv = q[:].rearrange("i c (hb s hw) w -> hb s c i (hw w)", s=2, hw=4)
    kv = k[:].rearrange("j c (hb s hw) w -> hb s c j (hw w)", s=2, hw=4)
    vv = v[:].rearrange("j c (hb g r) w -> hb g j c (r w)", g=4, r=2)
    ov = o[:].rearrange("i c (hb g r) w -> hb g i c (r w)", g=4, r=2)

    with tile.TileContext(nc) as tc, ExitStack() as ctx:
        io = ctx.enter_context(tc.tile_pool(name="io", bufs=2))
        sm = ctx.enter_context(tc.tile_pool(name="sm", bufs=3))
        ep = ctx.enter_context(tc.tile_pool(name="ep", bufs=2, space="PSUM"))
        op = ctx.enter_context(tc.tile_pool(name="op", bufs=2, space="PSUM"))

        chunk_tiles = {}

        def emit_chunk_dma(hb):
            tq = io.tile([128, 32, 4, W], F16, tag="tq")
            tk = io.tile([128, 32, 4, W], F16, tag="tk")
            tv = io.tile([128, 65, 2, W], F16, tag="tv")
            to = io.tile([128, C, 2, W], F16, tag="to")
            tqf = tq.rearrange("p i hw w -> p i (hw w)")
            tkf = tk.rearrange("p j hw w -> p j (hw w)")
            tvf = tv.rearrange("p c r w -> p c (r w)")
            for s in range(2):
                nc.sync.dma_start(out=tqf[64 * s : 64 * s + 64], in_=qv[hb, s])
                nc.sync.dma_start(out=tkf[64 * s : 64 * s + 64], in_=kv[hb, s])
            for g in range(4):
                nc.sync.dma_start(
                    out=tvf[32 * g : 32 * g + 32, 0:64], in_=vv[hb, g]
                )
            nc.gpsimd.memset(tv[:, 64], 1.0)
            chunk_tiles[hb] = (tq, tk, tv, to)

        def emit_chunk_out(hb):
            (_, _, _, to) = chunk_tiles.pop(hb)
            tof = to.rearrange("p c r w -> p c (r w)")
            for g in range(4):
                nc.scalar.dma_start(
                    out=ov[hb, g], in_=tof[32 * g : 32 * g + 32]
                )

        def emit_scores(sg):
            hb, r, wb = sg["hb"], sg["r"], sg["wb"]
            tq, tk, _, _ = chunk_tiles[hb]
            eb = ep.tile([128, 512], F32, tag="eb")
            for u in range(WB):
                w_ = WB * wb + u
                for g in range(4):
                    s, hw = g // 2, 2 * (g % 2) + r
                    nc.tensor.matmul(
                        out=eb[32 * g : 32 * g + 32, 32 * u : 32 * u + 32],
                        lhsT=tk[64 * s : 64 * s + 64, :, hw, w_],
                        rhs=tq[64 * s : 64 * s + 64, :, hw, w_],
                        start=True,
                        stop=True,
                        tile_position=(64 * s, 32 * g),
                    )
            sg["eb"] = eb

        def emit_softmax_delta(sg):
            hb, r, wb = sg["hb"], sg["r"], sg["wb"]
            _, _, tv, to = chunk_tiles[hb]
            eb = sg.pop("eb")
            E = sm.tile([128, 512], F16, tag="E")
            nc.scalar.activation(E, eb, mybir.ActivationFunctionType.Exp, scale=0.125)
            ob0 = op.tile([128, 462], F32, tag="ob0")
            ob1 = op.tile([128, 462], F32, tag="ob1")
            ob2 = op.tile([128, 132], F32, tag="ob2")
            obs = [ob0, ob1, ob2]
            for u in range(WB):
                w_ = WB * wb + u
                ob, uu = obs[min(u // 7, 2)], u - 7 * min(u // 7, 2)
                for g in range(4):
                    nc.tensor.matmul(
                        out=ob[32 * g : 32 * g + 32, 66 * uu : 66 * uu + 65],
                        lhsT=E[32 * g : 32 * g + 32, 32 * u : 32 * u + 32],
                        rhs=tv[32 * g : 32 * g + 32, :, r, w_],
                        start=True,
                        stop=True,
                        tile_position=(32 * g, 32 * g),
                    )
            rden = sm.tile([128, 16], F32, tag="rden")
            tow = to.rearrange("p c r w -> p w r c")
            for bi, nb in ((0, 7), (1, 7), (2, 2)):
                obv = obs[bi].rearrange("p (u c) -> p u c", c=66)
                u0 = 7 * bi
                nc.vector.reciprocal(rden[:, u0 : u0 + nb], obv[:, :, 64])
                nc.vector.tensor_mul(
                    tow[:, WB * wb + u0 : WB * wb + u0 + nb, r],
                    obv[:, :, 0:64],
                    rden[:, u0 : u0 + nb].unsqueeze(2).broadcast_to((128, nb, 64)),
                )

        sgs = [
            {"hb": hb, "r": r, "wb": wb}
            for hb in range(NCHUNK)
            for r in range(2)
            for wb in range(NSGW)
        ]
        emit_chunk_dma(0)
        pending = None
        for n, sg in enumerate(sgs):
            if sg["wb"] == 0 and sg["r"] == 0 and sg["hb"] + 1 < NCHUNK:
                emit_chunk_dma(sg["hb"] + 1)
            emit_scores(sg)
            if pending is not None:
                emit_softmax_delta(pending)
                if pending["wb"] == NSGW - 1 and pending["r"] == 1:
                    emit_chunk_out(pending["hb"])
            pending = sg
        emit_softmax_delta(pending)
        emit_chunk_out(pending["hb"])
    nc.compile()
    return nc


def _get_nc():
    if "nc" not in _CACHE:
        _CACHE["nc"] = build_nc()
    return _CACHE["nc"]


def kernel(q_big, k_big, v_big, **run_kwargs):
    nc = _get_nc()
    in_maps = []
    for core in range(8):
        b, h0 = core // 2, HPC * (core % 2)
        in_maps.append(
            {
                "q": np.asarray(q_big[:, b, :, h0 : h0 + HPC, :]).astype(np.float16),
                "k": np.asarray(k_big[:, b, :, h0 : h0 + HPC, :]).astype(np.float16),
                "v": np.asarray(v_big[:, b, :, h0 : h0 + HPC, :]).astype(np.float16),
            }
        )
    res = run_bass_kernel_spmd(nc, in_maps, list(range(8)), **run_kwargs)
    out = np.empty((NB, B, C, H, W), np.float32)
    for core in range(8):
        b, h0 = core // 2, HPC * (core % 2)
        out[:, b, :, h0 : h0 + HPC, :] = res.results[core]["o"].astype(np.float32)
    if run_kwargs:
        kernel.last_results = res
    return out


# revision 9
# speedup vs baseline: 4.2110x; 1.4341x over previous
"""Box-attention kernel for Trainium2 (Bass/Tile), SPMD over 8 NeuronCores.

Problem: per-(batch, h, w) pixel attention over 32 boxes:
  S[i,j] = <q[i,:,p], k[j,:,p]>/8 ; W = softmax_j S ; delta[i,c,p] = sum_j W[i,j] v[j,c,p]

Sharding: core = 2*b + h_half; each core owns (b = core//2, h in [40*(core%2), +40)).
Zero communication (pixel-parallel).

v2 design (vs v1 baseline at 1.245 ms):
 - fp16 I/O: host casts q,k,v to fp16 and the output back to fp32. Halves HBM
   traffic (52.4 MB/core); fp16 error (2^-11) is far inside the 2e-2 gate.
 - chunk = 8 h-rows; q/k SBUF [64*(hc//4)+c][i, hc%4, w] -> 640B DMA lines;
   v/out [32*(hc//2)+{j,i}][c, hc%2, w] -> 320B lines.
 - compact score banks: e-bank [128, 512] holds 64 pixels (4 part-groups x
   16 col-groups of 32) with zero padding waste. One exp per 64 px.
 - denominator fused into the delta matmul: v tile has a 65th channel == 1.0,
   so out[.., 64] = sum_j E[j,i] = denom. No ones-matmul, no [128,512]
   reciprocal. Reciprocal runs on [128,16] compact denominators instead
   (DVE iterative divide is ~6.5 cyc/elem -- was 53% of v1's span).
 - normalize+evacuate fused: one DVE tensor_mult per ob bank with rden
   broadcast (step-0) along c, writing fp16 straight into the staging tile.
 - PSUM: eb x2, (ob0,ob1,ob2) x2 = 8 banks exactly.
 - emission skew: scores(n+1) issued before delta(n) so the PE never waits
   on the ACT exp; engines pipeline across supergroups.
"""

import sys

import numpy as np

try:
    import concourse.bass as bass
except ImportError:  # fresh grading dir: point at the in-container repo
    for p in ("/opt/trn_rl_repo", "/root/.axon_site/_ro/trn_rl_repo"):
        if p not in sys.path:
            sys.path.insert(0, p)
    import concourse.bass as bass

from contextlib import ExitStack

import concourse.bacc as bacc
import concourse.tile as tile
from concourse import mybir
from concourse.bass_utils import run_bass_kernel_spmd

NB, B, C, H, W = 32, 4, 64, 80, 80
HPC = H // 2  # h rows per core (8 cores = 4 batches x 2 h-halves)
CHH = 8  # chunk height (h rows)
NCHUNK = HPC // CHH  # 5
WB = 16  # w values per supergroup (64 px = 4 groups x 16 cols)
NSGW = W // WB  # 5 supergroups per (chunk, r)
F16 = mybir.dt.float16
F32 = mybir.dt.float32

_CACHE = {}


def build_nc():
    # The host pre-permutes inputs into the exact SBUF layouts (see _pack_*),
    # so every DMA is [128 partitions x 20KB contiguous] - descriptor-perfect.
    # q/k: [hb][64*(hc//4)+c][i, (hc%4)*80+w]; v: [hb][32*(hc//2)+j][c, (hc%2)*80+w]
    # o (output) mirrors v with i instead of j.
    nc = bacc.Bacc()
    q = nc.declare_dram_parameter("q", [NCHUNK, 128, NB, 4 * W], F16, isOutput=False)
    k = nc.declare_dram_parameter("k", [NCHUNK, 128, NB, 4 * W], F16, isOutput=False)
    v = nc.declare_dram_parameter("v", [NCHUNK, 128, C, 2 * W], F16, isOutput=False)
    o = nc.declare_dram_parameter("o", [NCHUNK, 128, C, 2 * W], F16, isOutput=True)
    qv, kv, vv, ov = q[:], k[:], v[:], o[:]

    with tile.TileContext(nc) as tc, ExitStack() as ctx:
        io = ctx.enter_context(tc.tile_pool(name="io", bufs=2))
        sm = ctx.enter_context(tc.tile_pool(name="sm", bufs=3))
        ep = ctx.enter_context(tc.tile_pool(name="ep", bufs=2, space="PSUM"))
        op = ctx.enter_context(tc.tile_pool(name="op", bufs=2, space="PSUM"))

        chunk_tiles = {}

        def emit_chunk_dma(hb):
            tq = io.tile([128, 32, 4, W], F16, tag="tq")
            tk = io.tile([128, 32, 4, W], F16, tag="tk")
            tv = io.tile([128, 65, 2, W], F16, tag="tv")
            to = io.tile([128, C, 2, W], F16, tag="to")
            tqf = tq.rearrange("p i hw w -> p i (hw w)")
            tkf = tk.rearrange("p j hw w -> p j (hw w)")
            tvf = tv.rearrange("p c r w -> p c (r w)")
            nc.sync.dma_start(out=tqf, in_=qv[hb])
            nc.sync.dma_start(out=tkf, in_=kv[hb])
            nc.sync.dma_start(out=tvf[:, 0:64], in_=vv[hb])
            nc.gpsimd.memset(tv[:, 64], 1.0)
            chunk_tiles[hb] = (tq, tk, tv, to)

        def emit_chunk_out(hb):
            (_, _, _, to) = chunk_tiles.pop(hb)
            tof = to.rearrange("p c r w -> p c (r w)")
            nc.sync.dma_start(out=ov[hb], in_=tof)

        def emit_scores(sg):
            hb, r, wb = sg["hb"], sg["r"], sg["wb"]
            tq, tk, _, _ = chunk_tiles[hb]
            eb = ep.tile([128, 512], F32, tag="eb")
            for u in range(WB):
                w_ = WB * wb + u
                for g in range(4):
                    s, hw = g // 2, 2 * (g % 2) + r
                    nc.tensor.matmul(
                        out=eb[32 * g : 32 * g + 32, 32 * u : 32 * u + 32],
                        lhsT=tk[64 * s : 64 * s + 64, :, hw, w_],
                        rhs=tq[64 * s : 64 * s + 64, :, hw, w_],
                        start=True,
                        stop=True,
                        tile_position=(64 * s, 32 * g),
                    )
            sg["eb"] = eb

        def emit_softmax_delta(sg):
            hb, r, wb = sg["hb"], sg["r"], sg["wb"]
            _, _, tv, to = chunk_tiles[hb]
            eb = sg.pop("eb")
            E = sm.tile([128, 512], F16, tag="E")
            nc.scalar.activation(E, eb, mybir.ActivationFunctionType.Exp, scale=0.125)
            ob0 = op.tile([128, 462], F32, tag="ob0")
            ob1 = op.tile([128, 462], F32, tag="ob1")
            ob2 = op.tile([128, 132], F32, tag="ob2")
            obs = [ob0, ob1, ob2]
            for u in range(WB):
                w_ = WB * wb + u
                ob, uu = obs[min(u // 7, 2)], u - 7 * min(u // 7, 2)
                for g in range(4):
                    nc.tensor.matmul(
                        out=ob[32 * g : 32 * g + 32, 66 * uu : 66 * uu + 65],
                        lhsT=E[32 * g : 32 * g + 32, 32 * u : 32 * u + 32],
                        rhs=tv[32 * g : 32 * g + 32, :, r, w_],
                        start=True,
                        stop=True,
                        tile_position=(32 * g, 32 * g),
                    )
            # normalize + evacuate: out inner dim is w (unit-stride fp16
            # writes); the strided side is the PSUM read (c step 1, u step 66).
            rden = sm.tile([128, 16], F32, tag="rden")
            for bi, nb in ((0, 7), (1, 7), (2, 2)):
                obv = obs[bi].rearrange("p (u c) -> p c u", c=66)
                u0 = 7 * bi
                w0 = WB * wb + u0
                nc.vector.reciprocal(rden[:, u0 : u0 + nb], obv[:, 64])
                nc.vector.tensor_mul(
                    to[:, :, r, w0 : w0 + nb],
                    obv[:, 0:64],
                    rden[:, u0 : u0 + nb].unsqueeze(1).broadcast_to((128, 64, nb)),
                )

        sgs = [
            {"hb": hb, "r": r, "wb": wb}
            for hb in range(NCHUNK)
            for r in range(2)
            for wb in range(NSGW)
        ]
        emit_chunk_dma(0)
        pending = None
        for n, sg in enumerate(sgs):
            if sg["wb"] == 0 and sg["r"] == 0 and sg["hb"] + 1 < NCHUNK:
                emit_chunk_dma(sg["hb"] + 1)
            emit_scores(sg)
            if pending is not None:
                emit_softmax_delta(pending)
                if pending["wb"] == NSGW - 1 and pending["r"] == 1:
                    emit_chunk_out(pending["hb"])
            pending = sg
        emit_softmax_delta(pending)
        emit_chunk_out(pending["hb"])
    nc.compile()
    return nc


def _get_nc():
    if "nc" not in _CACHE:
        _CACHE["nc"] = build_nc()
    return _CACHE["nc"]


def _pack_qk(a):
    # [32, 64, 40, 80] fp32 -> [5, 128=(s c), 32 i, 320=(hw w)] fp16
    t = np.asarray(a).reshape(NB, C, NCHUNK, 2, 4, W).transpose(2, 3, 1, 0, 4, 5)
    return t.astype(np.float16).reshape(NCHUNK, 128, NB, 4 * W)


def _pack_v(a):
    # [32, 64, 40, 80] fp32 -> [5, 128=(g j), 64 c, 160=(r w)] fp16
    t = np.asarray(a).reshape(NB, C, NCHUNK, 4, 2, W).transpose(2, 3, 0, 1, 4, 5)
    return t.astype(np.float16).reshape(NCHUNK, 128, C, 2 * W)


def _unpack_o(oh):
    # [5, 128=(g i), 64 c, 160=(r w)] fp16 -> [32, 64, 40, 80] fp32
    t = oh.reshape(NCHUNK, 4, NB, C, 2, W).astype(np.float32)
    return t.transpose(2, 3, 0, 1, 4, 5).reshape(NB, C, HPC, W)


def kernel(q_big, k_big, v_big, **run_kwargs):
    nc = _get_nc()
    in_maps = []
    for core in range(8):
        b, h0 = core // 2, HPC * (core % 2)
        sl = np.s_[:, b, :, h0 : h0 + HPC, :]
        in_maps.append(
            {
                "q": _pack_qk(q_big[sl]),
                "k": _pack_qk(k_big[sl]),
                "v": _pack_v(v_big[sl]),
            }
        )
    res = run_bass_kernel_spmd(nc, in_maps, list(range(8)), **run_kwargs)
    out = np.empty((NB, B, C, H, W), np.float32)
    for core in range(8):
        b, h0 = core // 2, HPC * (core % 2)
        out[:, b, :, h0 : h0 + HPC, :] = _unpack_o(res.results[core]["o"])
    if run_kwargs:
        kernel.last_results = res
    return out
